# revision 17
# baseline (speedup 1.0000x reference)
"""MoE top-2/8 expert-parallel kernel for TRN2 (8 cores), v2.

Host pre-casts inputs to bf16 (x and router kernel as hi/lo splits for
fp32-accurate routing; expert weights plain bf16).

Per-core (core c == expert c) pipeline:
  1. DMA-xbar transposed x tiles (no PE/DVE transposes) -> router logits on
     own 512-token slice in split-bf16 (3-term matmul).
  2. AllGather logits (tiny, issued first), then AllGather bf16(x) (16MB) --
     both overlap the routing math below (collectives run on TOPSP).
  3. Vectorized top-2 (reduce_max + masked 2nd max, no DVE max8/max_index);
     w1 = sigmoid((E1-E2)/Z); per-expert compaction via triangular-matmul
     cumsum -> slot per token; slot lists via one scatter_add roundtrip.
  4. dma_gather(transpose=True) in 3 chunks of 384 tokens; per chunk:
     gate/up matmuls (bf16, fp32 accum) -> silu*up -> down matmul -> scale
     by gate weight -> dma_scatter_add into a dense [T(+trash), D] bf16 acc.
  5. ReduceScatter(bf16) over 8 cores -> own 512-token slice -> fp32 out
     via cast-DMA.
"""

import numpy as np
import concourse.bass as bass
import concourse.mybir as mybir
import concourse.tile as tile
from concourse import bacc
from concourse.masks import make_identity, make_upper_triangular

P = 128
T, D, F, E = 4096, 2048, 1024, 8
GC = 384             # gather chunk (3 chunks = C)
C = 3 * GC           # 1152 per-expert token capacity (measured max 1058)
NTRASH = 64          # trash rows / slots
dt = mybir.dt
AF = mybir.ActivationFunctionType
ALU = mybir.AluOpType

TO = T // P   # 32 token columns (t = o*128 + p)
KO = D // P   # 16 contraction tiles over D
FO = F // P   # 8 f-tiles
CM = C // P   # 9 slot tiles
C16 = C // 16           # 72
NIL = 1280              # ilist2 rows (C + NTRASH = 1216, padded to 10*128)


def build(n_cores: int = 8, repeat: int = 1, ablate: str | None = None):
    TS = T // n_cores
    SO = TS // P
    nc = bacc.Bacc("TRN2", target_bir_lowering=False, debug=False,
                   num_devices=n_cores)

    xh = nc.dram_tensor("xh", [TS, D], dt.bfloat16, kind="ExternalInput")
    xl = nc.dram_tensor("xl", [TS, D], dt.bfloat16, kind="ExternalInput")
    rkh = nc.dram_tensor("rkh", [D, E], dt.bfloat16, kind="ExternalInput")
    rkl = nc.dram_tensor("rkl", [D, E], dt.bfloat16, kind="ExternalInput")
    wg = nc.dram_tensor("wg", [D, F], dt.bfloat16, kind="ExternalInput")
    wu = nc.dram_tensor("wu", [D, F], dt.bfloat16, kind="ExternalInput")
    wd = nc.dram_tensor("wd", [F, D], dt.bfloat16, kind="ExternalInput")
    eid = nc.dram_tensor("eid", [P, 1], dt.float32, kind="ExternalInput")
    out = nc.dram_tensor("out", [TS, D], dt.float32, kind="ExternalOutput")

    with tile.TileContext(nc) as tc:
        with (
            tc.tile_pool(name="dram", bufs=1, space="DRAM") as dram,
            tc.tile_pool(name="consts", bufs=1) as consts,
            tc.tile_pool(name="wpool", bufs=1) as wpool,
            tc.tile_pool(name="main", bufs=1) as main,
        ):
          for _rep in range(repeat):
            # ---------------- DRAM scratch ----------------
            xh_int = dram.tile([TS, D], dt.bfloat16)
            lg_slice = dram.tile([TS, E], dt.float32)
            xbf_full = dram.tile([T, D], dt.bfloat16, addr_space="Shared")
            lg_full = dram.tile([T, E], dt.float32, addr_space="Shared")
            slotd = dram.tile([T], dt.int16)
            ilist2 = dram.tile([NIL, 64], dt.float32)
            acc = dram.tile([T + NTRASH, D], dt.bfloat16)
            rs_out = dram.tile([TS, D], dt.bfloat16)

            # ---------------- constants / input-independent ---------------
            ident_f32 = consts.tile([P, P], dt.float32)
            make_identity(nc, ident_f32[:])
            ident_bf = consts.tile([P, P], dt.bfloat16)
            make_identity(nc, ident_bf[:])
            triu_bf = consts.tile([P, P], dt.bfloat16)
            make_upper_triangular(nc, triu_bf[:], val=1.0, diag=True)
            eid_sb = consts.tile([P, 1], dt.float32)
            nc.sync.dma_start(eid_sb[:], eid[:])

            zero_sb = consts.tile([P, D], dt.bfloat16)
            nc.vector.memset(zero_sb[:], 0.0)
            zero_il = consts.tile([P, (NIL // P) * 64], dt.float32)
            nc.vector.memset(zero_il[:], 0.0)
            ones_bf = consts.tile([P, 1], dt.bfloat16)
            nc.vector.memset(ones_bf[:], 1.0)
            # ehot[p, o, e] = (e == expert_id) as f32
            ehot_i = consts.tile([P, TO, E], dt.int32)
            nc.gpsimd.iota(ehot_i[:], pattern=[[0, TO], [1, E]], base=0,
                           channel_multiplier=0)
            ehot = consts.tile([P, TO, E], dt.float32)
            nc.vector.tensor_copy(ehot[:], ehot_i[:])
            nc.vector.tensor_scalar(ehot[:], ehot[:], eid_sb[:], None,
                                    ALU.is_equal)
            trashv = consts.tile([P, 1], dt.int32)
            nc.gpsimd.iota(trashv[:], pattern=[[0, 1]], base=0,
                           channel_multiplier=1)
            nc.vector.tensor_scalar(trashv[:], trashv[:], 63, None,
                                    ALU.bitwise_and)
            trashf = consts.tile([P, 1], dt.float32)
            nc.vector.tensor_copy(trashf[:], trashv[:])
            nc.vector.tensor_scalar(trashf[:], trashf[:], float(C), None,
                                    ALU.add)
            # per-token payload rows [hi, lo, wsel, count]: 64 f32 = 256B
            tid32 = consts.tile([P, TO], dt.int32)
            nc.gpsimd.iota(tid32[:], pattern=[[P, TO]], base=0,
                           channel_multiplier=1)
            hi32 = consts.tile([P, TO], dt.int32)
            lo32 = consts.tile([P, TO], dt.int32)
            nc.vector.tensor_scalar(hi32[:], tid32[:], 6, None,
                                    ALU.arith_shift_right)
            nc.vector.tensor_scalar(lo32[:], tid32[:], 63, None,
                                    ALU.bitwise_and)
            rows = main.tile([P, TO, 64], dt.float32)
            nc.vector.memset(rows[:], 0.0)
            nc.vector.tensor_copy(rows[:, :, 0], hi32[:])
            nc.vector.tensor_copy(rows[:, :, 1], lo32[:])
            nc.vector.memset(rows[:, :, 3], 1.0)
            # init ilist2 rows to zero (one DMA)
            nc.sync.dma_start(
                ilist2[:].rearrange("(m p) c -> p m c", p=P),
                zero_il[:].rearrange("p (m c) -> p m c", m=NIL // P))
            # local copy of xh for the AllGather (collectives ban I/O tensors)
            for k in range(SO):
                nc.sync.dma_start(xh_int[k * P:(k + 1) * P, :],
                                  xh[k * P:(k + 1) * P, :])

            # ---------------- router ----------------
            with tc.tile_pool(name="route", bufs=1) as route, \
                 tc.tile_pool(name="ps_route", bufs=2, space="PSUM") as psr:
                rkh_sb = route.tile([P, KO, E], dt.bfloat16)
                rkl_sb = route.tile([P, KO, E], dt.bfloat16)
                nc.sync.dma_start(rkh_sb[:],
                                  rkh[:].rearrange("(ko p) e -> p ko e", p=P))
                nc.sync.dma_start(rkl_sb[:],
                                  rkl[:].rearrange("(ko p) e -> p ko e", p=P))
                # x tiles + PE transposes -> [d(128), TS]
                # (DMA-xbar transpose would serialize against the collectives)
                xh_sb = route.tile([P, SO, D], dt.bfloat16)
                xl_sb = route.tile([P, SO, D], dt.bfloat16)
                nc.sync.dma_start(xh_sb[:],
                                  xh[:].rearrange("(o p) d -> p o d", p=P))
                nc.sync.dma_start(xl_sb[:],
                                  xl[:].rearrange("(o p) d -> p o d", p=P))
                xhT = route.tile([P, KO, TS], dt.bfloat16)
                xlT = route.tile([P, KO, TS], dt.bfloat16)
                for src, dstT in ((xh_sb, xhT), (xl_sb, xlT)):
                    for ko in range(KO):
                        for o in range(SO):
                            pt = psr.tile([P, P], dt.bfloat16, tag="tp",
                                          name="pt")
                            nc.tensor.transpose(
                                pt[:], src[:, o, ko * P:(ko + 1) * P],
                                ident_bf[:])
                            nc.vector.tensor_copy(
                                dstT[:, ko, o * P:(o + 1) * P], pt[:])
                ps_l = psr.tile([E, TS], dt.float32, tag="psl", name="ps_l")
                steps = []
                for ko in range(KO):
                    steps.append((rkh_sb[:, ko], xhT[:, ko]))
                    steps.append((rkl_sb[:, ko], xhT[:, ko]))
                    steps.append((rkh_sb[:, ko], xlT[:, ko]))
                for i, (lhsT, rhs) in enumerate(steps):
                    nc.tensor.matmul(ps_l[:], lhsT, rhs, start=(i == 0),
                                     stop=(i == len(steps) - 1))
                lgT_sb = route.tile([E, TS], dt.float32)
                nc.vector.tensor_copy(lgT_sb[:], ps_l[:])
                lg_sb = route.tile([P, SO, E], dt.float32)
                for o in range(SO):
                    pt2 = psr.tile([P, E], dt.float32, tag="tp2", name="pt2")
                    nc.tensor.transpose(pt2[:], lgT_sb[:, o * P:(o + 1) * P],
                                        ident_f32[:E, :E])
                    nc.vector.tensor_copy(lg_sb[:, o], pt2[:])
                nc.sync.dma_start(
                    lg_slice[:].rearrange("(o p) e -> p o e", p=P), lg_sb[:])

                # ---- collectives: tiny logits AG first, then big x AG ----
                if n_cores > 1:
                    nc.gpsimd.collective_compute(
                        "AllGather", ALU.bypass,
                        ins=[lg_slice[:].opt()], outs=[lg_full[:].opt()],
                        replica_groups=[list(range(n_cores))])
                    nc.gpsimd.collective_compute(
                        "AllGather", ALU.bypass,
                        ins=[xh_int[:].opt()], outs=[xbf_full[:].opt()],
                        replica_groups=[list(range(n_cores))])
                else:
                    nc.sync.dma_start(lg_full[:], lg_slice[:])
                    nc.sync.dma_start(xbf_full[:], xh_int[:])

                # ---- heavy async loads, off the latency path ----
                # expert weights: bf16 from inputs, on the scalar HWDGE queue
                wg_sb = wpool.tile([P, KO, F], dt.bfloat16)
                wu_sb = wpool.tile([P, KO, F], dt.bfloat16)
                wd_sb = wpool.tile([P, FO, D], dt.bfloat16)
                wg_r = wg[:].rearrange("(ko p) f -> p ko f", p=P)
                wu_r = wu[:].rearrange("(ko p) f -> p ko f", p=P)
                wd_r = wd[:].rearrange("(fo p) d -> p fo d", p=P)
                for ko in range(KO):
                    nc.scalar.dma_start(wg_sb[:, ko], wg_r[:, ko])
                    nc.scalar.dma_start(wu_sb[:, ko], wu_r[:, ko])
                for fo in range(FO):
                    nc.scalar.dma_start(wd_sb[:, fo], wd_r[:, fo])
                # zero the dense accumulator (split over both HWDGE queues)
                for i, r0 in enumerate(range(0, T, P)):
                    eng = nc.scalar if i % 2 == 0 else nc.sync
                    eng.dma_start(acc[r0:r0 + P, :], zero_sb[:, :])

            # ---------------- routing math (vectorized top-2) -------------
            L = main.tile([P, TO, E], dt.float32)
            nc.sync.dma_start(L[:], lg_full[:].rearrange("(o p) e -> p o e", p=P))
            m1 = main.tile([P, TO], dt.float32)
            nc.vector.reduce_max(m1[:], L[:], axis=mybir.AxisListType.X)
            eq1 = main.tile([P, TO, E], dt.float32)
            for o in range(TO):
                nc.vector.tensor_scalar(eq1[:, o], L[:, o], m1[:, o:o + 1],
                                        None, ALU.is_equal)
            Lm = main.tile([P, TO, E], dt.float32)
            nc.vector.tensor_scalar(Lm[:], eq1[:], 1e30, None, ALU.mult)
            nc.vector.tensor_sub(Lm[:], L[:], Lm[:])
            m2 = main.tile([P, TO], dt.float32)
            nc.vector.reduce_max(m2[:], Lm[:], axis=mybir.AxisListType.X)
            # own expert's logit per token
            Lsel = main.tile([P, TO, E], dt.float32)
            nc.vector.tensor_mul(Lsel[:], L[:], ehot[:])
            Le = main.tile([P, TO], dt.float32)
            nc.vector.reduce_sum(Le[:], Lsel[:], axis=mybir.AxisListType.X)
            mask1 = main.tile([P, TO], dt.float32)
            mask2 = main.tile([P, TO], dt.float32)
            nc.vector.tensor_tensor(mask1[:], Le[:], m1[:], ALU.is_equal)
            nc.vector.tensor_tensor(mask2[:], Le[:], m2[:], ALU.is_equal)
            # w1 = sigmoid((E1-E2)/Z)
            expL = main.tile([P, TO, E], dt.float32)
            nc.scalar.activation(expL[:], L[:], AF.Exp)
            Z = main.tile([P, TO], dt.float32)
            nc.vector.reduce_sum(Z[:], expL[:], axis=mybir.AxisListType.X)
            E1 = main.tile([P, TO], dt.float32)
            E2 = main.tile([P, TO], dt.float32)
            nc.scalar.activation(E1[:], m1[:], AF.Exp)
            nc.scalar.activation(E2[:], m2[:], AF.Exp)
            rZ = main.tile([P, TO], dt.float32)
            nc.vector.reciprocal(rZ[:], Z[:])
            arg = main.tile([P, TO], dt.float32)
            nc.vector.tensor_sub(arg[:], E1[:], E2[:])
            nc.vector.tensor_mul(arg[:], arg[:], rZ[:])
            w1 = main.tile([P, TO], dt.float32)
            nc.scalar.activation(w1[:], arg[:], AF.Sigmoid)
            # wsel = mask2 + w1*(mask1 - mask2); mask = mask1 + mask2
            wsel = main.tile([P, TO], dt.float32)
            nc.vector.tensor_sub(wsel[:], mask1[:], mask2[:])
            nc.vector.tensor_mul(wsel[:], wsel[:], w1[:])
            nc.vector.tensor_add(wsel[:], wsel[:], mask2[:])
            mask = main.tile([P, TO], dt.float32)
            nc.vector.tensor_add(mask[:], mask1[:], mask2[:])
            nc.vector.tensor_copy(rows[:, :, 2], wsel[:])

            # ---------------- compaction: slot per token ------------------
            with tc.tile_pool(name="ps_cs", bufs=1, space="PSUM") as pscs_pool:
                maskb = main.tile([P, TO], dt.bfloat16)
                nc.vector.tensor_copy(maskb[:], mask[:])
                ps_cs = pscs_pool.tile([P, TO], dt.float32)
                nc.tensor.matmul(ps_cs[:], triu_bf[:], maskb[:], start=True,
                                 stop=True)
                csum = main.tile([P, TO], dt.float32)
                nc.vector.tensor_copy(csum[:], ps_cs[:])
                ps_tot = pscs_pool.tile([1, TO], dt.float32, name="ps_tot")
                nc.tensor.matmul(ps_tot[:], ones_bf[:], maskb[:], start=True,
                                 stop=True)
                coltot = main.tile([1, TO], dt.float32)
                nc.vector.tensor_copy(coltot[:], ps_tot[:])
            sc_a = main.tile([1, TO], dt.float32, tag="sca")
            sc_b = main.tile([1, TO], dt.float32, tag="scb")
            nc.vector.tensor_copy(sc_a[:], coltot[:])
            cur, nxt = sc_a, sc_b
            s = 1
            while s < TO:
                nc.vector.tensor_copy(nxt[:], cur[:])
                nc.vector.tensor_add(nxt[:, s:], cur[:, s:], cur[:, :TO - s])
                cur, nxt = nxt, cur
                s *= 2
            offs = main.tile([1, TO], dt.float32)
            nc.vector.memset(offs[:, 0:1], 0.0)
            nc.vector.tensor_copy(offs[:, 1:], cur[:, :TO - 1])
            offs_b = main.tile([P, TO], dt.float32)
            nc.gpsimd.partition_broadcast(offs_b[:], offs[:])

            pos = main.tile([P, TO], dt.float32)
            nc.vector.tensor_add(pos[:], csum[:], offs_b[:])
            nc.vector.tensor_sub(pos[:], pos[:], mask[:])
            slot = main.tile([P, TO], dt.float32)
            nc.vector.tensor_scalar(slot[:], pos[:], trashf[:], None,
                                    ALU.subtract)
            nc.vector.tensor_mul(slot[:], slot[:], mask[:])
            nc.vector.tensor_scalar(slot[:], slot[:], trashf[:], None, ALU.add)
            slot16 = main.tile([P, TO], dt.int16)
            nc.vector.tensor_copy(slot16[:], slot[:])

            # wrapped-by-16 slot list via DRAM roundtrip
            nc.sync.dma_start(slotd[:].rearrange("(o p) -> p o", p=P), slot16[:])
            slot16w = main.tile([P, T // 16], dt.int16)
            slotd_w = slotd[:].rearrange("(cw pw) -> pw cw", pw=16)
            for rep in range(8):
                nc.sync.dma_start(slot16w[rep * 16:(rep + 1) * 16, :], slotd_w)

            # scatter-add the payload rows into slot order
            for k in range(TO // 4):
                nc.gpsimd.dma_scatter_add(
                    out_ap=ilist2[:], in_ap=rows[:, 4 * k:4 * (k + 1), :],
                    idxs_ap=slot16w[:, k * 32:(k + 1) * 32],
                    num_idxs=4 * P, num_idxs_reg=4 * P, elem_size=64)

            # load back (wrapped-by-16 directly): tid per slot -> si/gi
            lbw = main.tile([P, C16, 4], dt.float32)
            lbw_src = ilist2[:C, :4].rearrange("(cw pw) c -> pw cw c", pw=16)
            for rep in range(8):
                nc.sync.dma_start(lbw[rep * 16:(rep + 1) * 16], lbw_src)
            tidf = main.tile([P, C16], dt.float32)
            nc.vector.tensor_scalar(tidf[:], lbw[:, :, 0], 64.0, None, ALU.mult)
            nc.vector.tensor_add(tidf[:], tidf[:], lbw[:, :, 1])
            emptyf = main.tile([P, C16], dt.float32)
            nc.vector.tensor_scalar(emptyf[:], lbw[:, :, 3], 1.0, None,
                                    ALU.subtract)
            nc.vector.tensor_scalar(emptyf[:], emptyf[:], float(-T), None,
                                    ALU.mult)
            nc.vector.tensor_add(tidf[:], tidf[:], emptyf[:])
            tclf = main.tile([P, C16], dt.float32)
            nc.vector.tensor_scalar(tclf[:], tidf[:], float(T - 1), None,
                                    ALU.min)
            sgi = main.tile([P, 2 * C16], dt.int16)
            nc.vector.tensor_copy(sgi[:, :C16], tidf[:])
            nc.vector.tensor_copy(sgi[:, C16:], tclf[:])
            si16 = sgi[:, :C16]
            # weight per slot in [P, CM] slot-tile layout (for down scaling)
            lb2 = main.tile([P, CM, 2], dt.float32)
            nc.sync.dma_start(
                lb2[:], ilist2[:C, 2:4].rearrange("(m p) c -> p m c", p=P))
            wlist = main.tile([P, CM], dt.float32)
            nc.vector.tensor_copy(wlist[:], lb2[:, :, 0])

            if ablate == "front":
                # keep routing/slot lists + both AGs live; skip MLP+RS
                nc.gpsimd.dma_start(out[:P, :CM], wlist[:])
                continue

            # ---------------- expert MLP: 3 pipelined chunks ---------------
            mmp = tc.tile_pool(name="mmp", bufs=1)
            mmpool = mmp.__enter__()
            xeTs = [mmpool.tile([P, KO, GC], dt.bfloat16, tag=f"xeT{k}",
                                name=f"xeT{k}") for k in range(3)]
            for k in range(3):
                nc.gpsimd.dma_gather(
                    out_ap=xeTs[k][:], in_ap=xbf_full[:],
                    idxs_ap=sgi[:, C16 + k * (GC // 16):C16 + (k + 1) * (GC // 16)],
                    num_idxs=GC, num_idxs_reg=GC, elem_size=D, transpose=True)

            fuse = mmpool.tile([P, FO, C], dt.bfloat16)
            dchunks = [(0, 512), (512, 512), (1024, 512), (1536, 512)]
            with tc.tile_pool(name="psgu", bufs=2, space="PSUM") as psgu, \
                 tc.tile_pool(name="psd", bufs=1, space="PSUM") as psd, \
                 tc.tile_pool(name="doutp", bufs=3) as doutp:
                for k in range(3):
                    for fo in range(FO):
                        gb = psgu.tile([P, GC], dt.float32, tag="g", name="g")
                        ub = psgu.tile([P, GC], dt.float32, tag="u", name="u")
                        for ko in range(KO):
                            nc.tensor.matmul(gb[:],
                                             wg_sb[:, ko, fo * P:(fo + 1) * P],
                                             xeTs[k][:, ko], start=(ko == 0),
                                             stop=(ko == KO - 1))
                        for ko in range(KO):
                            nc.tensor.matmul(ub[:],
                                             wu_sb[:, ko, fo * P:(fo + 1) * P],
                                             xeTs[k][:, ko], start=(ko == 0),
                                             stop=(ko == KO - 1))
                        sil = mmpool.tile([P, GC], dt.float32, tag="sil")
                        # silu(g)*u = g*sigmoid(g)*u (sim lacks Silu)
                        nc.scalar.activation(sil[:], gb[:], AF.Sigmoid)
                        nc.vector.tensor_mul(sil[:], sil[:], gb[:])
                        nc.vector.tensor_mul(fuse[:, fo, k * GC:(k + 1) * GC],
                                             sil[:], ub[:])
                    for j in range(3):
                        tm = k * 3 + j
                        dbank = [psd.tile([P, 512], dt.float32, tag=f"d{i}",
                                          name=f"d{i}") for i in range(4)]
                        for fo in range(FO):
                            for i, (d0, n) in enumerate(dchunks):
                                nc.tensor.matmul(dbank[i][:],
                                                 fuse[:, fo, tm * P:(tm + 1) * P],
                                                 wd_sb[:, fo, d0:d0 + n],
                                                 start=(fo == 0),
                                                 stop=(fo == FO - 1))
                        dout = doutp.tile([P, 1, D], dt.bfloat16, tag="dout")
                        for i, (d0, n) in enumerate(dchunks):
                            nc.vector.tensor_scalar(dout[:, 0, d0:d0 + n],
                                                    dbank[i][:],
                                                    wlist[:, tm:tm + 1], None,
                                                    ALU.mult)
                        if ablate == "no_scatter":
                            nc.sync.dma_start(acc[tm * P:(tm + 1) * P, :],
                                              dout[:, 0, :])
                        else:
                            nc.gpsimd.dma_scatter_add(
                                out_ap=acc[:], in_ap=dout[:],
                                idxs_ap=si16[:, tm * 8:(tm + 1) * 8],
                                num_idxs=P, num_idxs_reg=P, elem_size=D)

            if n_cores > 1 and ablate != "no_rs":
                nc.gpsimd.collective_compute(
                    "ReduceScatter", ALU.add,
                    ins=[acc[:T].opt()], outs=[rs_out[:].opt()],
                    replica_groups=[list(range(n_cores))])
            else:
                nc.sync.dma_start(rs_out[:], acc[:TS])
            mmp.__exit__(None, None, None)
            # final cast bf16 -> fp32 straight in DRAM via SWDGE cast-DMA
            nc.gpsimd.dma_start(out[:], rs_out[:])

    nc.compile()
    return nc


def make_in_maps(x, router_kernel, w_gate, w_up, w_down, n_cores=8):
    import ml_dtypes
    bf16 = ml_dtypes.bfloat16
    f32 = np.float32
    x = np.asarray(x, f32)
    rk = np.asarray(router_kernel, f32)
    xh = x.astype(bf16)
    xl = (x - xh.astype(f32)).astype(bf16)
    rkh = rk.astype(bf16)
    rkl = (rk - rkh.astype(f32)).astype(bf16)
    wgb = np.asarray(w_gate, f32).astype(bf16)
    wub = np.asarray(w_up, f32).astype(bf16)
    wdb = np.asarray(w_down, f32).astype(bf16)
    TS = T // n_cores
    in_maps = []
    for c in range(n_cores):
        in_maps.append({
            "xh": np.ascontiguousarray(xh[c * TS:(c + 1) * TS]),
            "xl": np.ascontiguousarray(xl[c * TS:(c + 1) * TS]),
            "rkh": rkh,
            "rkl": rkl,
            "wg": np.ascontiguousarray(wgb[c]),
            "wu": np.ascontiguousarray(wub[c]),
            "wd": np.ascontiguousarray(wdb[c]),
            "eid": np.full((P, 1), float(c), f32),
        })
    return in_maps


_NC_CACHE = {}


def _get_nc():
    if "nc" not in _NC_CACHE:
        _NC_CACHE["nc"] = build(n_cores=8)
    return _NC_CACHE["nc"]


def kernel(x, router_kernel, w_gate, w_up, w_down):
    """Full-input MoE forward on 8 TRN2 NeuronCores (expert-parallel)."""
    from concourse.bass_utils import run_bass_kernel_spmd

    nc = _get_nc()
    in_maps = make_in_maps(x, router_kernel, w_gate, w_up, w_down)
    res = run_bass_kernel_spmd(nc, in_maps, core_ids=list(range(8)))
    out = np.concatenate([res.results[c]["out"] for c in range(8)], axis=0)
    return out.astype(np.float32)
